# revision 1
# baseline (speedup 1.0000x reference)
"""CapsuleLayer dynamic-routing kernel for 8 TRN2 NeuronCores — v2.

Strategy: shard R(=8192) across the 8 cores (RL=1024). The routing recurrence
uses linearity of b_ij in v:  b(t) = u_hat . (v_0+...+v_{t-1}) = u . V.

Per routing pass, all heavy contractions run on the PE:
  h[b,c,r,i] = sum_o w[c,r,i,o] V[b,c,o]   (PE, K=128 w/ masked moving, out [r,b])
  logit[r,c,b] = sum_i x[r,i,b] * h[r,i,b] (DVE mul + add-tree, Pool assists)
  softmax over c in [r, c, b] layout       (ACT exp + strided adds)
  s[b,c,o] = sum_{k,i,r} y[r,i,b] w[r,o]   (PE, y=c_ij*x stationary, N=16,
                                            per-k psum groups + DVE k-sum)
Cross-core: s partials ([128,160] f32) AllReduced after passes 0/1; pass-2
partials are summed + squashed on host. V is transposed on-PE each pass into
four zero-masked moving operands Vt128[q] (rows 32q..32q+16 hold V^T), so all
h-matmuls share tile_position (0,0) — mixed tile positions into one psum tile
hang the device. Engine split: PE matmuls/transposes, ACT psum drains + exp,
DVE muls/softmax sums, Pool the last two add-tree levels.
"""
import numpy as np
import ml_dtypes
from contextlib import ExitStack

import concourse.bass as bass
import concourse.bacc as bacc
import concourse.tile as tile
from concourse import mybir
from concourse.bass_utils import run_bass_kernel_spmd

B, C, R, I, O = 128, 10, 8192, 8, 16
NCORES = 8
RL = R // NCORES          # 1024 r's per core
NK = 8                    # r-chunks of 128 per core
F32 = mybir.dt.float32
BF16 = mybir.dt.bfloat16
AX = mybir.AxisListType.X


def _bc(ap, dims):
    """Broadcast/permuted view of an AP. dims entries: int = reuse that free
    dim of ap, tuple (0, n) = broadcast dim of extent n."""
    new = []
    for d in dims:
        if isinstance(d, tuple):
            new.append([d[0], d[1]])
        else:
            new.append(ap.ap[d])
    return bass.AP(tensor=ap.tensor, offset=ap.offset, ap=new)


def _squash_emit(nc, pool, s_ap, v_out, bias_eps):
    """v_out[:, c, o] = squash(s_ap[:, c, o]) over o. All [128, C, O] f32."""
    m2 = pool.tile([B, C, O], F32, tag="sq_m2")
    nc.vector.tensor_mul(m2[:], s_ap, s_ap)
    sq = pool.tile([B, C], F32, tag="sq_sq")
    nc.vector.reduce_sum(out=sq[:], in_=m2[:], axis=AX)
    rt_ = pool.tile([B, C], F32, tag="sq_rt")
    nc.scalar.activation(rt_[:], sq[:], mybir.ActivationFunctionType.Sqrt,
                         bias=bias_eps[:], scale=1.0)
    d1 = pool.tile([B, C], F32, tag="sq_d1")
    nc.vector.tensor_scalar_add(d1[:], sq[:], 1.0)
    den = pool.tile([B, C], F32, tag="sq_den")
    nc.vector.tensor_mul(den[:], d1[:], rt_[:])
    rec = pool.tile([B, C], F32, tag="sq_rec")
    nc.vector.reciprocal(rec[:], den[:])
    scale = pool.tile([B, C], F32, tag="sq_scale")
    nc.vector.tensor_mul(scale[:], sq[:], rec[:])
    nc.vector.tensor_mul(v_out, s_ap, _bc(scale, [0, 1, (0, O)]))


def build_nc():
    nc = bacc.Bacc(None, num_devices=NCORES)
    x2_d = nc.declare_dram_parameter("x2", [128, NK * I * B], BF16, isOutput=False)
    whA_d = nc.declare_dram_parameter("whA", [128, C * NK * 128], BF16, isOutput=False)
    whB_d = nc.declare_dram_parameter("whB", [128, C * NK * 128], BF16, isOutput=False)
    ws_d = nc.declare_dram_parameter("ws", [128, NK * C * I * O], BF16, isOutput=False)
    id_d = nc.declare_dram_parameter("ident", [128, 128], BF16, isOutput=False)
    out_d = nc.declare_dram_parameter("s2", [B, C * O], F32, isOutput=True)

    with ExitStack() as ctx:
        tc = ctx.enter_context(tile.TileContext(nc))
        consts = ctx.enter_context(tc.tile_pool(name="consts", bufs=1))
        state = ctx.enter_context(tc.tile_pool(name="state", bufs=1))
        work = ctx.enter_context(tc.tile_pool(name="work", bufs=2))
        psum_h = ctx.enter_context(tc.tile_pool(name="psum_h", bufs=2, space="PSUM"))
        psum_s = ctx.enter_context(tc.tile_pool(name="psum_s", bufs=1, space="PSUM"))
        psum_v = ctx.enter_context(tc.tile_pool(name="psum_v", bufs=1, space="PSUM"))
        dram = ctx.enter_context(tc.tile_pool(name="dram", bufs=1, space="DRAM"))

        # ---- resident inputs ------------------------------------------------
        X2 = consts.tile([128, NK, I, B], BF16)           # x  [r, k, i, b]
        nc.sync.dma_start(out=X2[:], in_=x2_d[:].rearrange(
            "p (k i b) -> p k i b", i=I, b=B))
        WS = consts.tile([128, NK, C, I, O], BF16)        # w  [r, k, c, i, o]
        nc.sync.dma_start(out=WS[:], in_=ws_d[:].rearrange(
            "p (k c i o) -> p k c i o", c=C, i=I, o=O))
        WHA = consts.tile([128, C, NK, 128], BF16)        # w  [(i<4,o), c, k, r]
        WHB = consts.tile([128, C, NK, 128], BF16)        # w  [(i>=4,o), c, k, r]
        IDT = consts.tile([128, 128], BF16)
        nc.sync.dma_start(out=IDT[:], in_=id_d[:])

        bias_eps = state.tile([B, 1], F32)
        nc.vector.memset(bias_eps[:], 1e-8)
        bias_zero = state.tile([128, 1], F32)
        nc.vector.memset(bias_zero[:], 0.0)

        V = state.tile([B, C, O], F32)        # running sum of v_t
        sfull = state.tile([B, C, O], F32)    # AllReduced s
        Vb32 = state.tile([B, C, 32], BF16)   # V in bf16, o padded to 32 (zeros)
        nc.vector.memset(Vb32[:], 0.0)
        # V^T as 4 masked moving operands: Vt128[q] is zero except rows
        # [32q, 32q+16) which hold V^T[o, c, b]. Lets every h-matmul run at
        # K=128 / tile_position (0,0) (mixed tile positions into one psum
        # tile hang the device).
        Vt128 = []
        for q in range(4):
            vt_q = state.tile([128, C, B], BF16, tag=f"vt128_{q}")
            nc.vector.memset(vt_q[:], 0.0)
            Vt128.append(vt_q)

        def allreduce(idx, src, gate_wh=False):
            # bf16 payload: the collective's cost is (const + bytes/bw)*1.875,
            # so halving the 80KB payload saves ~1.9us per exchange. The
            # 8-way sum in bf16 costs ~0.2% relative error on s.
            ar_in = dram.tile([B, C * O], BF16, tag=f"ar_in{idx}")
            ar_out = dram.tile([B, C * O], BF16, tag=f"ar_out{idx}",
                               addr_space="Shared")
            nc.gpsimd.dma_start(out=ar_in[:],
                                in_=src[:].rearrange("b c o -> b (c o)"))
            if gate_wh:
                nc.gpsimd.dma_start(out=WHA[0:1, 0, 0, 0:1], in_=ar_in[0:1, 0:1])
                nc.sync.dma_start(out=WHA[:], in_=whA_d[:].rearrange(
                    "p (c k r) -> p c k r", k=NK, r=128))
                nc.sync.dma_start(out=WHB[:], in_=whB_d[:].rearrange(
                    "p (c k r) -> p c k r", k=NK, r=128))
            nc.gpsimd.collective_compute(
                "AllReduce", mybir.AluOpType.add,
                replica_groups=[list(range(NCORES))],
                ins=[ar_in[:].opt()], outs=[ar_out[:].opt()])
            nc.gpsimd.dma_start(out=sfull[:].rearrange("b c o -> b (c o)"),
                                in_=ar_out[:])

        def build_vt():
            """Vb32 <- bf16(V) ; Vt128[q][32q+o, c, b] <- V[b, c, o]."""
            nc.scalar.copy(Vb32[:, :, 0:O], V[:])
            for cp in range(C // 2):
                vt_ps = psum_v.tile([128, 2, 128], BF16, tag="vt_ps")
                for q in range(4):
                    for cc in range(2):
                        nc.tensor.transpose(
                            vt_ps[32 * q:32 * q + 32, cc, :],
                            Vb32[:, 2 * cp + cc, :], IDT[:],
                            tile_position=(0, 32 * q))
                for q in range(4):
                    nc.vector.tensor_copy(
                        Vt128[q][32 * q:32 * q + 32, 2 * cp:2 * cp + 2, :],
                        vt_ps[32 * q:32 * q + 32, :, :])

        # ---------------- pass 0: s0 = sum_r u (uniform c folded on host) ----
        s_sb0 = work.tile([B, C, O], F32, tag="s_sb0")
        for kk in range(2):
            s_ps0 = psum_s.tile([B, C, O], F32, tag="s_kps", bufs=2)
            for c in range(C):
                for k in range(4 * kk, 4 * kk + 4):
                    for i in range(I):
                        nc.tensor.matmul(
                            s_ps0[:, c, :], X2[:, k, i, :], WS[:, k, c, i, :],
                            start=(k == 4 * kk and i == 0),
                            stop=(k == 4 * kk + 3 and i == I - 1),
                            tile_position=(0, 0))
            if kk == 0:
                nc.scalar.copy(s_sb0[:], s_ps0[:])
            else:
                nc.vector.tensor_add(s_sb0[:], s_sb0[:], s_ps0[:])
        allreduce(0, s_sb0, gate_wh=True)
        # v0 = squash(0.1 * sfull)
        nc.vector.tensor_scalar_mul(sfull[:], sfull[:], 1.0 / C)
        _squash_emit(nc, work, sfull[:], V[:], bias_eps)
        build_vt()

        # ---------------- routing passes 1 and 2 -----------------------------
        for t in (1, 2):
            s_acc = work.tile([B, C, O], F32, tag=f"s_acc{t}")
            for k in range(NK):
                blog = work.tile([128, C, B], F32, tag="blog")
                for c in range(C):
                    h_ps = psum_h.tile([128, I, B], F32, tag="h_ps")
                    for i in range(I):
                        WH = WHA if i < 4 else WHB
                        q = i % 4
                        nc.tensor.matmul(
                            h_ps[:, i, :], WH[:, c, k, :],
                            Vt128[q][:, c, :],
                            start=True, stop=True, tile_position=(0, 0))
                    hs = work.tile([128, I, B], BF16, tag="hs", bufs=6)
                    nc.scalar.copy(hs[:], h_ps[:])
                    P = work.tile([128, I, B], BF16, tag="P", bufs=4)
                    nc.vector.tensor_mul(P[:], hs[:], X2[:, k])
                    t4 = work.tile([128, 4, B], BF16, tag="t4", bufs=4)
                    nc.vector.tensor_add(t4[:], P[:, 0:4], P[:, 4:8])
                    t2 = work.tile([128, 2, B], BF16, tag="t2", bufs=4)
                    nc.gpsimd.tensor_add(t2[:], t4[:, 0:2], t4[:, 2:4])
                    nc.gpsimd.tensor_add(blog[:, c, :], t2[:, 0, :], t2[:, 1, :])
                e = work.tile([128, C, B], BF16, tag="e")
                nc.scalar.activation(e[:], blog[:],
                                     mybir.ActivationFunctionType.Exp,
                                     bias=bias_zero[:], scale=1.0)
                u1 = work.tile([128, 5, B], BF16, tag="u1")
                nc.vector.tensor_add(u1[:], e[:, 0:5], e[:, 5:10])
                u2 = work.tile([128, 2, B], BF16, tag="u2")
                nc.vector.tensor_add(u2[:], u1[:, 0:2], u1[:, 2:4])
                u3 = work.tile([128, B], BF16, tag="u3")
                nc.vector.tensor_add(u3[:], u2[:, 0, :], u2[:, 1, :])
                dsum = work.tile([128, B], F32, tag="dsum")
                nc.vector.tensor_add(dsum[:], u3[:], u1[:, 4, :])
                rec = work.tile([128, B], F32, tag="rec")
                nc.vector.reciprocal(rec[:], dsum[:])
                recb = work.tile([128, B], BF16, tag="recb")
                nc.vector.tensor_copy(recb[:], rec[:])
                en = work.tile([128, C, B], BF16, tag="en", bufs=2)
                nc.vector.tensor_mul(en[:], e[:], _bc(recb[:], [0, (0, C), 1]))
                # y[c, i] = c_ij * x for all c in one op, then accumulate s
                y = work.tile([128, C, I, B], BF16, tag="y", bufs=2)
                nc.vector.tensor_mul(y[:, 0:8], _bc(en[:, 0:8, :], [0, 1, (0, I), 2]),
                                     _bc(X2[:, k], [0, (0, 8), 1, 2]))
                for cq in (8, 9):
                    nc.gpsimd.tensor_mul(y[:, cq], _bc(en[:, cq, :], [0, (0, I), 1]),
                                         X2[:, k])
                s_kps = psum_s.tile([B, C, O], F32, tag="s_kps", bufs=2)
                for c in range(C):
                    for i in range(I):
                        nc.tensor.matmul(
                            s_kps[:, c, :], y[:, c, i, :], WS[:, k, c, i, :],
                            start=(i == 0), stop=(i == I - 1),
                            tile_position=(0, 0))
                if k == 0:
                    nc.scalar.copy(s_acc[:], s_kps[:])
                else:
                    nc.vector.tensor_add(s_acc[:], s_acc[:], s_kps[:])
            if t == 1:
                allreduce(1, s_acc)
                v1 = work.tile([B, C, O], F32, tag="v1")
                _squash_emit(nc, work, sfull[:], v1[:], bias_eps)
                nc.vector.tensor_add(V[:], V[:], v1[:])
                build_vt()
            else:
                nc.sync.dma_start(out=out_d[:],
                                  in_=s_acc[:].rearrange("b c o -> b (c o)"))
    nc.compile()
    return nc


def _prep_shards(x, w):
    """Swizzle per-core shards into the PE-friendly layouts (bf16)."""
    bf = ml_dtypes.bfloat16
    ident = np.eye(128, dtype=bf)
    maps = []
    for core in range(NCORES):
        r0 = core * RL
        xs = x[:, r0:r0 + RL, :]                          # [B, RL, I]
        x2 = xs.reshape(B, NK, 128, I).transpose(2, 1, 3, 0)   # [r, k, i, b]
        wsl = w[:, r0:r0 + RL].reshape(C, NK, 128, I, O)  # [c, k, r, i, o]
        wio = wsl.transpose(3, 4, 0, 1, 2)                # [i, o, c, k, r]
        whA = np.zeros((4, 32, C, NK, 128), dtype=bf)
        whA[:, :O] = wio[0:4]
        whB = np.zeros((4, 32, C, NK, 128), dtype=bf)
        whB[:, :O] = wio[4:8]
        ws = wsl.transpose(2, 1, 0, 3, 4)                 # [r, k, c, i, o]
        maps.append({
            "x2": np.ascontiguousarray(x2.astype(bf)).reshape(128, NK * I * B),
            "whA": whA.reshape(128, C * NK * 128),
            "whB": whB.reshape(128, C * NK * 128),
            "ws": np.ascontiguousarray(ws.astype(bf)).reshape(128, NK * C * I * O),
            "ident": ident,
        })
    return maps


_NC_CACHE = {}


def kernel(x, route_weights, _trace=False):
    x = np.asarray(x, dtype=np.float32)
    w = np.asarray(route_weights, dtype=np.float32)
    in_maps = _prep_shards(x, w)
    if "nc" not in _NC_CACHE:
        _NC_CACHE["nc"] = build_nc()
    nc = _NC_CACHE["nc"]
    res = run_bass_kernel_spmd(nc, in_maps, core_ids=list(range(NCORES)))
    s2 = np.zeros((B, C * O), dtype=np.float32)
    for i in range(NCORES):
        s2 += np.asarray(res.results[i]["s2"], dtype=np.float32)
    s2 = s2.reshape(B, C, O)
    sq = np.sum(s2 * s2, axis=-1, keepdims=True)
    v = (sq / (1.0 + sq)) * s2 / np.sqrt(sq + 1e-8)
    return v.astype(np.float32)

